# revision 148
# baseline (speedup 1.0000x reference)
"""Trainium2 Bass kernel for nn_CausalSelfAttention (quartet-gated, row-normed).

Sharding: head-parallel across 8 cores (2 heads/core, both batches). Each core
computes its head-slice projections, scores, softmax, AV, and a partial
y @ Wo.T over its 128-wide C-slice; host sums the 8 fp16 partials.

Score pipeline (2 elementwise passes instead of 3):
  center k, k2 per head  ->  a' = q.k_c and b' = q2.k2_c are row-mean-free,
  so row-norm is a pure per-row scale and
    scores = c1 * [(b' + d) o a']   with per-row scalars
    c1 = m*qs*rsB*rsA,  d = (1-m)/(m*qs) * sigmaB.
  One DVE scalar_tensor_tensor builds X = (b'+d) o a'; the c1 scale and -4
  bias ride free on the Act Exp op.  Row variances come from the G-trick
  (G = K_c K_c^T, ex2 = q^T G q) so only causal score blocks are computed.
q2/k2 projections run as fp8e4 DoubleRow matmuls (2x PE): host pre-scales
W2 by 64 to stay in fp8 normal range; the 1/64 folds into copy-out scales.
"""

import sys

sys.path.insert(0, "/opt/trn_rl_repo")

import math

import ml_dtypes
import numpy as np
import concourse.bass as bass
import concourse.mybir as mybir
import concourse.tile as tile
from concourse.bass_utils import run_bass_kernel_spmd

DT = mybir.dt
AF = mybir.ActivationFunctionType
OP = mybir.AluOpType
PM = mybir.MatmulPerfMode

B = 2
T = 1024
C = 1024
H = 16
DH = 64
NCORES = 8
H2 = 2  # heads per core
BT = B * T
NB = T // 128  # 8 tq/tk blocks
SCALE = 1.0 / 8.0  # 1/sqrt(Dh)
W8SCALE = 64.0  # host pre-scale on Wq2/Wk2 before fp8 cast
EXP_BIAS = -4.0
MASKVAL = -30000.0


def _split_multi_waits(nc):
    """This walrus build accepts at most one sync-wait per instruction; hoist
    extras onto preceding wait-only EventSemaphore instructions."""
    n = 0
    for func in nc.m.functions:
        for block in func.blocks:
            new_insts = []
            for inst in block.instructions:
                si = inst.sync_info
                if si is not None and len(si.on_wait) > 1:
                    waits = list(si.on_wait)
                    for w in waits[:-1]:
                        n += 1
                        new_insts.append(
                            mybir.InstEventSemaphore(
                                name=f"waitsplit-{n}",
                                engine=inst.engine,
                                sync_info=mybir.SyncInfo(on_wait=[w], on_update=[]),
                            )
                        )
                    inst.sync_info = mybir.SyncInfo(
                        on_wait=[waits[-1]], on_update=list(si.on_update)
                    )
                new_insts.append(inst)
            block.instructions[:] = new_insts


def _build_program():
    nc = bass.Bass("TRN2", target_bir_lowering=False, debug=False, num_devices=NCORES)

    f8 = DT.float8e4
    f16 = DT.float16
    f32 = DT.float32

    xT_d = nc.dram_tensor("xT", [C, BT], f16, kind="ExternalInput").ap()
    xb8_d = nc.dram_tensor("xb8", [128, 8, BT], f8, kind="ExternalInput").ap()
    w_d = {
        p: nc.dram_tensor(f"w{p}", [C, 128], f16, kind="ExternalInput").ap()
        for p in ("q", "k", "v")
    }
    w8_d = {
        p: nc.dram_tensor(f"w{p}", [C, 128], f8, kind="ExternalInput").ap()
        for p in ("q2", "k2")
    }
    woT_d = nc.dram_tensor("woT", [128, C], f16, kind="ExternalInput").ap()
    trineg_d = nc.dram_tensor("trineg", [128, 128], f16, kind="ExternalInput").ap()
    id128_d = nc.dram_tensor("id128", [128, 128], f16, kind="ExternalInput").ap()
    # [128,1] f32 host consts: exp bias, ln(m*qs), ln((1-m)/(m*qs))
    expb_d = nc.dram_tensor("expb", [128, 1], f32, kind="ExternalInput").ap()
    c1sc_d = nc.dram_tensor("c1sc", [128, 1], f32, kind="ExternalInput").ap()
    ddsc_d = nc.dram_tensor("ddsc", [128, 1], f32, kind="ExternalInput").ap()
    out_d = nc.dram_tensor("out", [BT, C], f16, kind="ExternalOutput").ap()

    from contextlib import ExitStack

    with tile.TileContext(nc) as tc, ExitStack() as es:
        consts = es.enter_context(tc.tile_pool(name="consts", bufs=1))
        projp = es.enter_context(tc.tile_pool(name="projp", bufs=1))
        xpool = es.enter_context(tc.tile_pool(name="xpool", bufs=1))
        natp = es.enter_context(tc.tile_pool(name="natp", bufs=2))
        statp = es.enter_context(tc.tile_pool(name="statp", bufs=2))
        etp = es.enter_context(tc.tile_pool(name="etp", bufs=2))
        workp = es.enter_context(tc.tile_pool(name="workp", bufs=4))
        yp = es.enter_context(tc.tile_pool(name="yp", bufs=1))
        outp = es.enter_context(tc.tile_pool(name="outp", bufs=6))
        # PSUM bank budget (8 banks of 2KB/part): proj+wo 2, scores 4, stats 1,
        # g/z/av shared 1
        ps_proj = es.enter_context(tc.tile_pool(name="ps_proj", bufs=2, space="PSUM"))
        ps_sc = es.enter_context(tc.tile_pool(name="ps_sc", bufs=2, space="PSUM"))
        ps_misc = es.enter_context(tc.tile_pool(name="ps_misc", bufs=1, space="PSUM"))

        # ---- loads, ordered so batch-0 projections can start ASAP:
        # wk -> xch b0 -> wk2+xb8 b0 -> wq,wv,wq2 -> batch-1 x -> misc
        wts = {}
        for p in ("k", "q", "v"):
            wts[p] = consts.tile([128, 8, 128], f16, tag=f"w{p}", name=f"w{p}")
        for p in ("k2", "q2"):
            wts[p] = consts.tile([128, 8, 128], f8, tag=f"w{p}", name=f"w{p}")
        xch = {}
        xb8 = {}
        for b in range(B):
            for kc in range(8):
                xch[(b, kc)] = xpool.tile(
                    [128, T], f16, tag=f"x{b}_{kc}", name=f"x{b}_{kc}"
                )
            xb8[b] = xpool.tile([128, 8, T], f8, tag=f"xb8_{b}", name=f"xb8_{b}")

        def load_w(p):
            src = w_d[p] if p in ("k", "q", "v") else w8_d[p]
            nc.sync.dma_start(
                out=wts[p], in_=src.rearrange("(kc p) m -> p kc m", p=128)
            )

        def load_x(b):
            for kc in range(8):
                eng = nc.sync if kc % 2 == 0 else nc.scalar
                eng.dma_start(
                    out=xch[(b, kc)],
                    in_=xT_d[kc * 128 : (kc + 1) * 128, b * T : (b + 1) * T],
                )

        load_w("k")
        load_x(0)
        load_w("k2")
        for kc in range(8):
            eng = nc.gpsimd if kc % 2 == 0 else nc.vector
            eng.tensor_copy(xb8[0][:, kc, :], xch[(0, kc)])
        load_w("q")
        load_w("q2")
        load_w("v")
        load_x(1)
        for kc in range(8):
            nc.gpsimd.tensor_copy(xb8[1][:, kc, :], xch[(1, kc)])
        expb = consts.tile([128, 1], f32, tag="expb", name="expb")
        nc.sync.dma_start(out=expb, in_=expb_d)
        c1sc = consts.tile([128, 1], f32, tag="c1sc", name="c1sc")
        nc.sync.dma_start(out=c1sc, in_=c1sc_d)
        ddsc = consts.tile([128, 1], f32, tag="ddsc", name="ddsc")
        nc.sync.dma_start(out=ddsc, in_=ddsc_d)
        trineg = consts.tile([128, 128], f16, tag="trineg", name="trineg")
        nc.sync.dma_start(out=trineg, in_=trineg_d)
        id128 = consts.tile([128, 128], f16, tag="id128", name="id128")
        nc.sync.dma_start(out=id128, in_=id128_d)
        woT = consts.tile([128, C], f16, tag="woT", name="woT")
        nc.sync.dma_start(out=woT, in_=woT_d)
        ones16 = consts.tile([128, 1], f16, tag="ones16", name="ones16")
        nc.vector.memset(ones16, 1.0)
        onesrow = consts.tile([1, 128], f16, tag="onesrow", name="onesrow")
        nc.vector.memset(onesrow, 1.0)

        # ---- per-batch state ----
        projT = {}  # (b, p) -> [128, T] fp16 (p in q, v)
        kc_t = {}  # (b, mat) -> centered [128, T] fp16 (mat in k, k2)
        kbsum = {}  # (b, mat) -> [128, 1] f32 accum of copy-out
        y_b = {}
        for b in range(B):
            y_b[b] = yp.tile([128, 8, 128], f16, tag=f"y_{b}", name=f"y_{b}")

        def emit_proj(b):
            # order: k, k2 first (feed the stats chain), then q, v, q2
            for p in ("k", "k2", "q", "q2", "v"):
                if p in ("k", "k2"):
                    dst = workp.tile([128, T], f16, tag="X", name=f"{p}raw_{b}")
                    acc2 = statp.tile(
                        [128, 2], f32, tag=f"kbs_{p}_{b}", name=f"kbs_{p}_{b}"
                    )
                    kbsum[(b, p)] = acc2
                else:
                    dst = projp.tile([128, T], f16, tag=f"{p}_{b}", name=f"{p}_{b}")
                projT[(b, p)] = dst
                for n in range(2):
                    pps = ps_proj.tile([128, 512], f32, tag="proj_ps", name="proj_ps")
                    if p in ("q2", "k2"):
                        for j in range(4):
                            nc.tensor.matmul(
                                pps,
                                wts[p][:, 2 * j : 2 * j + 2, :],
                                xb8[b][:, 2 * j : 2 * j + 2, n * 512 : (n + 1) * 512],
                                start=(j == 0),
                                stop=(j == 3),
                                perf_mode=PM.DoubleRow,
                            )
                    else:
                        for kc in range(8):
                            nc.tensor.matmul(
                                pps,
                                wts[p][:, kc, :],
                                xch[(b, kc)][:, n * 512 : (n + 1) * 512],
                                start=(kc == 0),
                                stop=(kc == 7),
                            )
                    half = dst[:, n * 512 : (n + 1) * 512]
                    if p in ("k", "k2"):
                        sc = 1.0 if p == "k" else 1.0 / W8SCALE
                        nc.scalar.activation(
                            half, pps, AF.Copy, scale=sc,
                            accum_out=kbsum[(b, p)][:, n : n + 1],
                        )
                    elif p == "q":
                        nc.vector.tensor_scalar_mul(half, pps, SCALE)
                    elif p == "v":
                        nc.scalar.copy(half, pps)
                    else:  # q2
                        nc.vector.tensor_scalar_mul(half, pps, SCALE / W8SCALE)

        def emit_center(b):
            for mat in ("k", "k2"):
                kb1 = statp.tile(
                    [128, 1], f32, tag=f"kb1_{mat}_{b}", name=f"kb1_{mat}_{b}"
                )
                nc.vector.tensor_reduce(
                    kb1, kbsum[(b, mat)], axis=mybir.AxisListType.X, op=OP.add
                )
                kbsc = statp.tile(
                    [128, 1], f32, tag=f"kbsc_{mat}_{b}", name=f"kbsc_{mat}_{b}"
                )
                nc.scalar.activation(kbsc, kb1, AF.Copy, scale=1.0 / T)
                cen = projp.tile([128, T], f16, tag=f"{mat}c_{b}", name=f"{mat}c_{b}")
                nc.vector.tensor_scalar(
                    cen, projT[(b, mat)], kbsc, None, op0=OP.subtract
                )
                kc_t[(b, mat)] = cen

        pairs = [(b, h) for b in range(B) for h in range(H2)]
        nat = {}
        stats = {}

        def emit_stats(b):
            """Row-variance stats for BOTH heads of batch b at once
            (z/ws span the full 128 partitions)."""
            # nat transposes (DMA): k_c, k2_c for G — full 128 rows
            for mat, src in (
                ("k", kc_t[(b, "k")]),
                ("k2", kc_t[(b, "k2")]),
            ):
                dst = natp.tile(
                    [128, 8, 128], f16, tag=f"nat_{mat}", name=f"nat_{mat}_{b}"
                )
                nc.sync.dma_start_transpose(dst, src)
                nat[(b, mat)] = dst
            # v nat with a ones column per head: [v_h | 1] so the AV matmul
            # also produces softmax row-sums in its last column
            vn = natp.tile([128, 8, 128], f16, tag="nat_vr", name=f"nat_vr_{b}")
            nc.sync.dma_start_transpose(vn, projT[(b, "v")])
            v1 = natp.tile([128, 8, 132], f16, tag="nat_v", name=f"nat_v_{b}")
            nc.vector.memset(v1, 1.0)  # ones cols 64/65 per head survive
            nc.gpsimd.tensor_copy(v1[:, :, 0:64], vn[:, :, 0:64])
            nc.gpsimd.tensor_copy(v1[:, :, 66:130], vn[:, :, 64:128])
            nat[(b, "v")] = v1

            sts = {}
            for mi, (mat, qn) in enumerate((("k", "q"), ("k2", "q2"))):
                st_ps = ps_misc.tile(
                    [128, 16], f32, tag="st_ps", bufs=1, name="st_ps"
                )
                kn = nat[(b, mat)]
                g_s = statp.tile([128, 64], f16, tag=f"g_{mi}", name=f"g_{b}_{mat}")
                for h in range(H2):
                    hs = slice(h * 64, h * 64 + 64)
                    g_ps = ps_misc.tile(
                        [64, 64], f32, tag="gza_ps", bufs=1, name="g_ps"
                    )
                    for j in range(8):
                        nc.tensor.matmul(
                            g_ps,
                            kn[:, j, hs],
                            kn[:, j, hs],
                            start=(j == 0),
                            stop=(j == 7),
                        )
                    nc.scalar.activation(g_s[hs, :], g_ps, AF.Copy, scale=1.0 / T)
                # z = G q (both heads), ws = z o q, ex2_h = colsums of ws[hs]
                qt = projT[(b, qn)]
                ws = workp.tile([128, T], f16, tag="ws", name="ws")
                for n in range(2):
                    z_ps = ps_misc.tile(
                        [128, 512], f32, tag="gza_ps", bufs=1, name="z_ps"
                    )
                    for h in range(H2):
                        hs = slice(h * 64, h * 64 + 64)
                        nc.tensor.matmul(
                            z_ps[hs, :],
                            g_s[hs, :],
                            qt[hs, n * 512 : (n + 1) * 512],
                        )
                    nc.vector.tensor_tensor(
                        ws[:, n * 512 : (n + 1) * 512],
                        z_ps,
                        qt[:, n * 512 : (n + 1) * 512],
                        op=OP.mult,
                    )
                for m in range(NB):
                    nc.tensor.matmul(
                        st_ps[:, 2 * m : 2 * m + 2],
                        ws[:, m * 128 : (m + 1) * 128],
                        ones2,
                    )
                stv = statp.tile([128, 16], f32, tag=f"sts_{mi}", name=f"sts_{b}_{mi}")
                nc.scalar.copy(stv, st_ps)
                sts[mat] = stv  # cols h*8+m = ex2/T for head h, block m

            for h in range(H2):
                pi = pairs.index((b, h))
                cA = sts["k"][:, h * 8 : h * 8 + 8]
                cB = sts["k2"][:, h * 8 : h * 8 + 8]
                vAB = statp.tile([128, 8], f32, tag="vAB", name=f"vAB_{pi}")
                nc.vector.tensor_tensor(vAB, cA, cB, op=OP.mult)
                rAB = statp.tile([128, 8], f32, tag="rAB", name=f"rAB_{pi}")
                nc.vector.reciprocal(rAB, vAB)
                c1f = statp.tile([128, 8], f16, tag=f"c1_{pi}", name=f"c1_{pi}")
                nc.scalar.activation(c1f, rAB, AF.Sqrt, scale=c1sc)
                ddf = statp.tile([128, 8], f16, tag=f"dd_{pi}", name=f"dd_{pi}")
                nc.scalar.activation(ddf, cB, AF.Sqrt, scale=ddsc)
                # row-vector forms [1, T]: transpose [128,8] -> [8,128], then
                # DMA-reshape into a single partition row
                cdT_ps = ps_misc.tile(
                    [8, 256], f16, tag="st_ps", bufs=1, name="cdT_ps"
                )
                nc.tensor.transpose(cdT_ps[:, 0:128], c1f, id128)
                nc.tensor.transpose(cdT_ps[:, 128:256], ddf, id128)
                c1T = statp.tile([8, 128], f16, tag=f"c1T_{pi}", name=f"c1T_{pi}")
                nc.vector.tensor_copy(c1T, cdT_ps[:, 0:128])
                ddT = statp.tile([8, 128], f16, tag=f"ddT_{pi}", name=f"ddT_{pi}")
                nc.scalar.copy(ddT, cdT_ps[:, 128:256])
                c1row = statp.tile([1, T], f16, tag=f"c1r_{pi}", name=f"c1r_{pi}")
                nc.sync.dma_start(out=c1row, in_=c1T)
                # q2x: [q2_h ; d-row], k2x: [k2c_h ; ones] -> the +d term rides
                # the B matmul as a 65th contraction row (no extra PE cycles)
                q2x = projp.tile([65, T], f16, tag=f"q2x_{pi}", name=f"q2x_{pi}")
                hsl = slice(h * 64, h * 64 + 64)
                nc.vector.tensor_copy(q2x[0:64, :], projT[(b, "q2")][hsl, :])
                nc.sync.dma_start(out=q2x[64:65, :], in_=ddT)
                k2x = projp.tile([65, T], f16, tag=f"k2x_{pi}", name=f"k2x_{pi}")
                nc.gpsimd.tensor_copy(k2x[0:64, :], kc_t[(b, "k2")][hsl, :])
                nc.vector.memset(k2x[64:65, :], 1.0)
                stats[pi] = dict(q2x=q2x, k2x=k2x, c1row=c1row)
            # qc1 = q o broadcast(c1): rank-1 matmuls (ones x c1T-row) into
            # PSUM broadcast c1 across partitions, one TT per half applies it
            qc1 = projp.tile([128, T], f16, tag=f"qc1_{b}", name=f"qc1_{b}")
            for n in range(2):
                bc_ps = ps_sc.tile([128, 512], f32, tag="sc_a", bufs=2, name="bc_ps")
                for h in range(H2):
                    pi = pairs.index((b, h))
                    nc.tensor.matmul(
                        bc_ps[h * 64 : h * 64 + 64, :],
                        onesrow[:, 0:64],
                        stats[pi]["c1row"][:, n * 512 : (n + 1) * 512],
                    )
                nc.vector.tensor_tensor(
                    qc1[:, n * 512 : (n + 1) * 512],
                    bc_ps,
                    projT[(b, "q")][:, n * 512 : (n + 1) * 512],
                    op=OP.mult,
                )
            for h in range(H2):
                stats[pairs.index((b, h))]["qc1"] = qc1

        escore_eT = {}

        def emit_scores(pi, fuse_av=False):
            b, h = pairs[pi]
            hs = slice(h * 64, h * 64 + 64)
            st = stats[pi]
            kt = kc_t[(b, "k")]

            e_T = etp.tile([128, 8, T], f16, tag="e_T", name=f"e_T_{pi}")
            for kcb in range(NB):
                t0 = kcb * 128
                width = T - t0
                nch = (width + 511) // 512
                X = workp.tile([128, T], f16, tag="X", name="X")
                for n in range(nch):
                    c0 = t0 + n * 512
                    nn = min(512, T - c0)
                    b_ps = ps_sc.tile([128, 512], f32, tag="sc_b", bufs=2, name="b_ps")
                    a_ps = ps_sc.tile([128, 512], f32, tag="sc_a", bufs=2, name="a_ps")
                    nc.tensor.matmul(
                        b_ps[:, :nn],
                        st["k2x"][:, t0 : t0 + 128],
                        st["q2x"][:, c0 : c0 + nn],
                    )
                    nc.tensor.matmul(
                        a_ps[:, :nn],
                        kt[hs, t0 : t0 + 128],
                        st["qc1"][hs, c0 : c0 + nn],
                    )
                    # pass 1: tb = b'' (d already folded in via the extra row)
                    if (kcb + n) % 3 != 0:
                        nc.scalar.copy(X[:, c0 : c0 + nn], b_ps[:, :nn])
                    else:
                        nc.vector.tensor_copy(X[:, c0 : c0 + nn], b_ps[:, :nn])
                    # pass 2: X = a'' o tb in place
                    nc.vector.tensor_tensor(
                        X[:, c0 : c0 + nn],
                        a_ps[:, :nn],
                        X[:, c0 : c0 + nn],
                        op=OP.mult,
                    )
                # causal mask on the diagonal block (tq < tk)
                nc.gpsimd.tensor_tensor(
                    X[:, t0 : t0 + 128],
                    X[:, t0 : t0 + 128],
                    trineg,
                    op=OP.add,
                )
                nc.scalar.activation(
                    e_T[:, kcb, t0:T],
                    X[:, t0:T],
                    AF.Exp,
                    bias=expb,
                )
                if fuse_av:
                    escore_eT[pi] = e_T
                    emit_av_m2(pi, kcb)
            escore_eT[pi] = e_T

        def emit_av_m2(pi, m):
            b, h = pairs[pi]
            e_T = escore_eT[pi]
            if True:
                # AV with ones-column rhs: col 64 = softmax row-sum
                av_ps = ps_misc.tile(
                    [128, 66], f32,
                    tag="gza_ps" if (m + pi) % 2 == 0 else "st_ps",
                    bufs=1, name="av_ps",
                )
                for kcb in range(m + 1):
                    nc.tensor.matmul(
                        av_ps,
                        e_T[:, kcb, m * 128 : (m + 1) * 128],
                        nat[(b, "v")][:, kcb, h * 66 : h * 66 + 66],
                        start=(kcb == 0),
                        stop=(kcb == m),
                    )
                recip = statp.tile([128, 1], f32, tag="recip", name=f"recip_{pi}_{m}")
                nc.vector.reciprocal(recip, av_ps[:, 64:65])
                dst = y_b[b][:, m, h * 64 : h * 64 + 64]
                if m % 2 == 0:
                    nc.scalar.activation(
                        dst, av_ps[:, 0:64], AF.Copy, scale=recip
                    )
                else:
                    nc.vector.tensor_scalar_mul(dst, av_ps[:, 0:64], recip)

        def emit_av(pi):
            for m in range(NB):
                emit_av_m2(pi, m)

        def emit_wo_m(b, yT, m):
            if True:
                o_sb = outp.tile([128, C], f16, tag="o_sb", name="o_sb")
                for n in range(2):
                    wo_ps = ps_proj.tile(
                        [128, 512], f32, tag="proj_ps", name="wo_ps"
                    )
                    nc.tensor.matmul(
                        wo_ps,
                        yT[:, m * 128 : (m + 1) * 128],
                        woT[:, n * 512 : (n + 1) * 512],
                    )
                    dst = o_sb[:, n * 512 : (n + 1) * 512]
                    if (m + n) % 2 == 0:
                        nc.scalar.copy(dst, wo_ps)
                    else:
                        nc.vector.tensor_copy(dst, wo_ps)
                nc.sync.dma_start(
                    out=out_d[b * T + m * 128 : b * T + (m + 1) * 128, :],
                    in_=o_sb,
                )

        def emit_wo(b):
            yT = yp.tile([128, T], f16, tag=f"yT_{b}", name=f"yT_{b}")
            nc.sync.dma_start_transpose(
                yT.rearrange("p (j f) -> p j f", j=8), y_b[b]
            )
            for m in range(NB):
                emit_wo_m(b, yT, m)

        # ---- emission order: interleave batches for cross-phase overlap ----
        emit_proj(0)
        emit_center(0)
        emit_proj(1)
        emit_stats(0)
        emit_center(1)
        emit_stats(1)
        emit_scores(0)
        emit_av(0)
        emit_scores(1)
        yT0 = yp.tile([128, T], f16, tag="yT_0", name="yT_0")
        for half in range(2):
            for m in range(half * 4, half * 4 + 4):
                emit_av_m2(1, m)
            nc.sync.dma_start_transpose(
                yT0[:, half * 512 : (half + 1) * 512].rearrange(
                    "p (j f) -> p j f", j=4
                ),
                y_b[0][:, half * 4 : half * 4 + 4, :],
            )
            for m in range(half * 4, half * 4 + 4):
                emit_wo_m(0, yT0, m)
        emit_scores(2)
        emit_av(2)
        emit_scores(3)
        yT1 = yp.tile([128, T], f16, tag="yT_1", name="yT_1")
        for half in range(2):
            for m in range(half * 4, half * 4 + 4):
                emit_av_m2(3, m)
            nc.sync.dma_start_transpose(
                yT1[:, half * 512 : (half + 1) * 512].rearrange(
                    "p (j f) -> p j f", j=4
                ),
                y_b[1][:, half * 4 : half * 4 + 4, :],
            )
            for m in range(half * 4, half * 4 + 4):
                emit_wo_m(1, yT1, m)

    _split_multi_waits(nc)
    return nc


_NC_CACHE = None
LAST_RESULT = None


def _make_in_maps(inputs):
    x = np.asarray(inputs["x"], np.float32)
    Wq = np.asarray(inputs["Wq"], np.float32)
    Wk = np.asarray(inputs["Wk"], np.float32)
    Wv = np.asarray(inputs["Wv"], np.float32)
    Wq2 = np.asarray(inputs["Wq2"], np.float32)
    Wk2 = np.asarray(inputs["Wk2"], np.float32)
    Wo = np.asarray(inputs["Wo"], np.float32)
    mixture = np.asarray(inputs["mixture"], np.float32)
    quartet_scale = np.asarray(inputs["quartet_scale"], np.float32)

    m = 1.0 / (1.0 + np.exp(-float(mixture[0])))
    mqs = m * float(quartet_scale[0])
    homq = (1.0 - m) / mqs
    adj = float(T) / (T - 1)
    expb = np.full((128, 1), EXP_BIAS, np.float32)
    c1sc = np.full((128, 1), (mqs / adj) * (mqs / adj), np.float32)
    ddsc = np.full((128, 1), adj * homq * homq, np.float32)

    xT = np.ascontiguousarray(x.reshape(BT, C).T).astype(np.float16)
    # xb8[p, kc, b*T+t] = x[b, t, kc*128+p]
    xb8 = np.ascontiguousarray(
        xT.reshape(8, 128, BT).transpose(1, 0, 2)
    ).astype(ml_dtypes.float8_e4m3)
    trineg = ((np.triu(np.ones((128, 128))) - 1.0) * -MASKVAL).astype(np.float16)

    in_maps = []
    for c in range(NCORES):
        cs = slice(c * 128, (c + 1) * 128)
        in_maps.append(
            {
                "xT": xT,
                "xb8": xb8,
                "wq": np.ascontiguousarray(Wq[cs, :].T).astype(np.float16),
                "wk": np.ascontiguousarray(Wk[cs, :].T).astype(np.float16),
                "wv": np.ascontiguousarray(Wv[cs, :].T).astype(np.float16),
                "wq2": np.ascontiguousarray(Wq2[cs, :].T * W8SCALE).astype(
                    ml_dtypes.float8_e4m3
                ),
                "wk2": np.ascontiguousarray(Wk2[cs, :].T * W8SCALE).astype(
                    ml_dtypes.float8_e4m3
                ),
                "woT": np.ascontiguousarray(Wo[:, cs].T).astype(np.float16),
                "trineg": trineg,
                "id128": np.eye(128, dtype=np.float16),
                "expb": expb,
                "c1sc": c1sc,
                "ddsc": ddsc,
            }
        )

    return in_maps


def kernel(**inputs) -> np.ndarray:
    global _NC_CACHE
    in_maps = _make_in_maps(inputs)
    if _NC_CACHE is None:
        _NC_CACHE = _build_program()
    res = run_bass_kernel_spmd(_NC_CACHE, in_maps, core_ids=list(range(NCORES)))
    global LAST_RESULT
    LAST_RESULT = res
    out = np.zeros((BT, C), np.float32)
    for c in range(NCORES):
        out += res.results[c]["out"].astype(np.float32)
    return out.reshape(B, T, C)


if __name__ == "__main__":
    rng = np.random.default_rng(0)
    ins = {
        "x": rng.standard_normal((B, T, C)).astype(np.float32),
        "Wq": rng.standard_normal((C, C)).astype(np.float32) * 0.02,
        "Wk": rng.standard_normal((C, C)).astype(np.float32) * 0.02,
        "Wv": rng.standard_normal((C, C)).astype(np.float32) * 0.02,
        "Wq2": rng.standard_normal((C, C)).astype(np.float32) * 0.02,
        "Wk2": rng.standard_normal((C, C)).astype(np.float32) * 0.02,
        "Wo": rng.standard_normal((C, C)).astype(np.float32) * 0.02,
        "mixture": np.full((1,), -5.0, np.float32),
        "quartet_scale": np.ones((1,), np.float32),
    }
    y = kernel(**ins)
    print("out", y.shape, y.dtype, float(np.abs(y).max()))
